# revision 2
# baseline (speedup 1.0000x reference)
"""BitLinear (ternary-weight + 8-bit-activation quantized matmul) on 8 TRN2 cores.

Strategy: data-parallel over tokens. Each core gets 2048 of the 16384 tokens
plus the full weight matrix, computes the whole BitLinear forward for its
token shard on device, and the host concatenates the shards.

Math (must match the jax reference):
  w_scale = max(mean(|W|), 1e-6)                       (scalar)
  w_q     = clip(round(W / w_scale), -1, 1)            (ternary)
  a       = clip(max_i |x|, 1e-8, inf)                 (per token)
  x_q     = clip(round(x * 127 / a), -127, 127)        (8-bit ints)
  y       = (x_q @ w_q^T) * w_scale * a / 127

All rounding is done with the fp32 magic-number trick (v + 1.5*2^23 - 1.5*2^23
is round-to-nearest-even), so device results bit-match jnp.round. x_q (ints
<= 127) and w_q ({-1,0,1}) are exact in bf16 and products accumulate exactly
in fp32 PSUM (|y_int| < 2^24), so the bf16 TensorE matmul is exact.
"""

from contextlib import ExitStack

import numpy as np

import concourse.bass as bass
import concourse.tile as tile
from concourse import bacc, bass_isa, mybir
from concourse.bass import ds, ts
from concourse.bass_utils import run_bass_kernel_spmd

F32 = mybir.dt.float32
BF16 = mybir.dt.bfloat16
AF = mybir.ActivationFunctionType
OP = mybir.AluOpType
AX = mybir.AxisListType

B, S, D_IN, D_OUT = 4, 4096, 2048, 2048
N_CORES = 8
TOK = B * S                # 16384 tokens
TPC = TOK // N_CORES       # 2048 tokens per core
NT = TPC // 128            # 16 token tiles per core
NJ = D_OUT // 128          # 16 weight row tiles
NI = D_IN // 128           # 16 contraction (k) blocks
NO = D_OUT // 512          # 4 output column blocks
CM = 12582912.0            # 1.5 * 2^23: fp32 RNE rounding magic
QMAX = 127.0

# Scheduling knobs (tuned via TimelineSim; see test notes).
KNOBS = {
    "ld_bufs": 6,
    "ldx_bufs": 2,
    "t1_bufs": 3,
    "inplace_t1": False,
    "abs_on_act": True,
    "shard_pass1": True,
    "wq_bufs": 4,
    "xqt_bufs": 3,
    "ys_bufs": 2,
    "pref": 3,
    "tpose_ring": "scalar",   # engine issuing DMA transposes
    "w2_batch": 4,            # pass-2: quantize N tiles, then N transposes
}

_CACHE = {}


def _emit(tc: tile.TileContext, x_d: bass.AP, w_d: bass.AP, ws_d: bass.AP, y_d: bass.AP):
    nc = tc.nc
    with ExitStack() as ctx:
        ld = ctx.enter_context(tc.tile_pool(name="ld", bufs=KNOBS["ld_bufs"]))
        ldx = ctx.enter_context(tc.tile_pool(name="ldx", bufs=KNOBS["ldx_bufs"]))
        t1p = (
            ctx.enter_context(tc.tile_pool(name="t1p", bufs=KNOBS["t1_bufs"]))
            if not KNOBS["inplace_t1"]
            else None
        )
        wqp = ctx.enter_context(tc.tile_pool(name="wqp", bufs=KNOBS["wq_bufs"]))
        xqp = ctx.enter_context(tc.tile_pool(name="xqp", bufs=2))
        xqtp = ctx.enter_context(tc.tile_pool(name="xqtp", bufs=KNOBS["xqt_bufs"]))
        wqtp = ctx.enter_context(tc.tile_pool(name="wqtp", bufs=1))
        ysp = ctx.enter_context(tc.tile_pool(name="ysp", bufs=KNOBS["ys_bufs"]))
        stats = ctx.enter_context(tc.tile_pool(name="stats", bufs=5))
        consts = ctx.enter_context(tc.tile_pool(name="consts", bufs=1))
        wsp = ctx.enter_context(tc.tile_pool(name="wsp", bufs=1))
        psum = ctx.enter_context(
            tc.tile_pool(name="psum", bufs=8, space=bass.MemorySpace.PSUM)
        )
        dram = ctx.enter_context(
            tc.tile_pool(name="dram", bufs=2, space=bass.MemorySpace.DRAM)
        )

        tpose_eng = nc.scalar if KNOBS["tpose_ring"] == "scalar" else nc.sync
        cpos = consts.tile([128, 1], F32, tag="cpos")
        nc.vector.memset(cpos, CM)

        # ---- W pass 1: abs-sum over the weight matrix ----
        # Sharded mode: each core reduces only its own 1/8 of the rows (a
        # separate per-core "ws" input) and the partial [128,1] sums are
        # AllReduce-added across the 8 cores via a DRAM bounce. Cuts the
        # serial pre-scale W read from 16.8 MB to 2.1 MB per core.
        # On the Scalar engine (Abs + accum_out row-sum) so the DVE is free
        # for the x-prep and pass-2 quantization that overlap this phase.
        czero = consts.tile([128, 1], F32, tag="czero")
        nc.vector.memset(czero, 0.0)
        npass1 = NJ // N_CORES if KNOBS["shard_pass1"] else NJ
        wsums = wsp.tile([128, npass1], F32, tag="wsums")
        for j in range(npass1):
            wt = ld.tile([128, D_IN], F32, tag="ld")
            src = ws_d if KNOBS["shard_pass1"] else w_d
            nc.sync.dma_start(wt, src[ts(j, 128), :])
            if KNOBS["abs_on_act"]:
                nc.scalar.activation(
                    wt, wt, AF.Abs, bias=czero, accum_out=wsums[:, ds(j, 1)]
                )
            else:
                nc.vector.reduce_sum(
                    wsums[:, ds(j, 1)], wt, axis=AX.X, apply_absolute_value=True
                )

        wsum_p = stats.tile([128, 1], F32, tag="wsp")
        if npass1 > 1:
            nc.vector.reduce_sum(wsum_p, wsums, axis=AX.X)
        else:
            nc.vector.tensor_copy(wsum_p, wsums)
        if KNOBS["shard_pass1"]:
            cin = dram.tile([128, 1], F32, tag="cin")
            cout = dram.tile([128, 1], F32, tag="cout")
            nc.scalar.dma_start(cin, wsum_p)
            nc.gpsimd.collective_compute(
                "AllReduce",
                OP.add,
                replica_groups=[list(range(N_CORES))],
                ins=[cin.opt()],
                outs=[cout.opt()],
            )
            wsum_x = stats.tile([128, 1], F32, tag="wsx")
            nc.scalar.dma_start(wsum_x, cout)
        else:
            wsum_x = wsum_p
        wsum_all = stats.tile([128, 1], F32, tag="wsa")
        nc.gpsimd.partition_all_reduce(wsum_all, wsum_x, 128, bass_isa.ReduceOp.add)
        # w_scale = max(sum / (O*I), 1e-6); long-lived -> consts pool
        wscale = consts.tile([128, 1], F32, tag="wscale")
        nc.vector.tensor_scalar(
            wscale, wsum_all, 1.0 / (D_OUT * D_IN), 1e-6, OP.mult, OP.max
        )
        # rws ~= 1/w_scale with one Newton refinement
        r0 = stats.tile([128, 1], F32, tag="wr0")
        nc.vector.reciprocal(r0, wscale)
        ntt = stats.tile([128, 1], F32, tag="wntt")
        nc.vector.tensor_mul(ntt, wscale, r0)
        nc.vector.tensor_scalar(ntt, ntt, -1.0, 2.0, OP.mult, OP.add)
        rws = consts.tile([128, 1], F32, tag="rws")
        nc.vector.tensor_mul(rws, r0, ntt)
        ws127 = consts.tile([128, 1], F32, tag="ws127")
        nc.vector.tensor_scalar(ws127, wscale, 1.0 / QMAX, None, OP.mult)

        # x-side prep chain: load, per-token scales, quantize, transpose.
        # Bulk loads/stores ride the sync HWDGE ring; DMA transposes ride the
        # scalar HWDGE ring (a transpose pays an xbar-mode drain against
        # in-flight copy DMAs, so keep them off the load ring's critical path).
        def x_prep(t):
            xt = ldx.tile([128, D_IN], F32, tag="ldx", name=f"xt{t}")
            nc.sync.dma_start(xt, x_d[ts(t, 128), :])
            a = stats.tile([128, 1], F32, tag="xa", name=f"xa{t}")
            nc.vector.reduce_max(a, xt, axis=AX.X, apply_absolute_value=True)
            nc.vector.tensor_scalar(a, a, 1e-8, None, OP.max)
            r0 = stats.tile([128, 1], F32, tag="xr0", name=f"xr0{t}")
            nc.vector.reciprocal(r0, a)
            ntt = stats.tile([128, 1], F32, tag="xntt", name=f"xntt{t}")
            nc.vector.tensor_mul(ntt, a, r0)
            nc.vector.tensor_scalar(ntt, ntt, -1.0, 2.0, OP.mult, OP.add)
            s = stats.tile([128, 1], F32, tag="xs", name=f"xs{t}")
            nc.vector.tensor_mul(s, r0, ntt)
            nc.vector.tensor_scalar(s, s, QMAX, None, OP.mult)  # 127/a
            sout = stats.tile([128, 1], F32, tag="xsout", name=f"xsout{t}")
            nc.vector.tensor_mul(sout, a, ws127)  # a * w_scale / 127

            if KNOBS["inplace_t1"]:
                t1 = xt
            else:
                t1 = t1p.tile([128, D_IN], F32, tag="t1", name=f"xt1_{t}")
            nc.scalar.activation(t1, xt, AF.Identity, bias=cpos, scale=s)
            xq = xqp.tile([128, D_IN], BF16, tag="xq", name=f"xq{t}")
            nc.vector.tensor_scalar(xq, t1, -CM, None, OP.add)
            # one-shot transpose: xqT[p, b, t'] = xq[t', b*128+p]
            xqT = xqtp.tile([128, NI, 128], BF16, tag="xqT", name=f"xqT{t}")
            tpose_eng.dma_start(xqT, xq, transpose=True)
            return xqT, sout

        # Prefetch the first token tiles' quantization so the GEMM can start
        # the moment the first wqT quarter lands.
        PREF = KNOBS["pref"]
        prefetched = [x_prep(t) for t in range(PREF)]

        # ---- W pass 2: quantize + transpose ----
        # One quarter tile per 512-wide output block so the GEMM can start as
        # soon as its own four j-tiles are quantized (whole-tile deps would
        # otherwise stall the first matmul on the last wqT write).
        # wqT[no][i_in, jq, i_blk, o_in] = w_q[(no*4+jq)*128 + o_in, i_blk*128 + i_in]
        wqT = [
            wqtp.tile(
                [128, NJ // NO, NI, 128], BF16, tag=f"wqT{no}", name=f"wqT{no}"
            )
            for no in range(NO)
        ]
        WB = KNOBS["w2_batch"]
        for j0 in range(0, NJ, WB):
            wqs = []
            for j in range(j0, min(j0 + WB, NJ)):
                wt = ld.tile([128, D_IN], F32, tag="ld", name=f"wt2_{j}")
                nc.sync.dma_start(wt, w_d[ts(j, 128), :])
                if KNOBS["inplace_t1"]:
                    t1 = wt
                else:
                    t1 = t1p.tile([128, D_IN], F32, tag="t1", name=f"wt1_{j}")
                # t1 = W * rws + CM  (fp32 add at ulp=1 == RNE round)
                nc.scalar.activation(t1, wt, AF.Identity, bias=cpos, scale=rws)
                # clip in the offset domain: min(max(t1, CM-1), CM+1)
                nc.vector.tensor_scalar(
                    t1, t1, CM - 1.0, CM + 1.0, OP.max, OP.min
                )
                wq = wqp.tile([128, D_IN], BF16, tag=f"wqn{j % KNOBS['wq_bufs']}", name=f"wq{j}", bufs=1)
                nc.vector.tensor_scalar(wq, t1, -CM, None, OP.add)
                wqs.append((j, wq))
            for j, wq in wqs:
                tpose_eng.dma_start(
                    wqT[j // 4][:, j % 4, :, :], wq, transpose=True
                )

        # ---- main loop over token tiles ----
        for t in range(NT):
            if t < PREF:
                xqT, sout = prefetched[t]
            else:
                xqT, sout = x_prep(t)

            ys = ysp.tile([128, D_OUT], F32, tag="ys")
            for no in range(NO):
                ps = psum.tile([128, 512], F32, tag="ps")
                for b in range(NI):
                    nc.tensor.matmul(
                        ps,
                        xqT[:, b, :],
                        wqT[no][:, :, b, :],
                        start=(b == 0),
                        stop=(b == NI - 1),
                    )
                nc.vector.tensor_scalar(
                    ys[:, ts(no, 512)], ps, sout, None, OP.mult
                )
            nc.sync.dma_start(y_d[ts(t, 128), :], ys)


def _build():
    key = tuple(sorted(KNOBS.items()))
    if key in _CACHE:
        return _CACHE[key]
    nc = bacc.Bacc(
        "TRN2", target_bir_lowering=False, debug=False, num_devices=N_CORES
    )
    x_d = nc.dram_tensor("x", [TPC, D_IN], F32, kind="ExternalInput").ap()
    w_d = nc.dram_tensor("w", [D_OUT, D_IN], F32, kind="ExternalInput").ap()
    ws_d = nc.dram_tensor(
        "ws", [D_OUT // N_CORES, D_IN], F32, kind="ExternalInput"
    ).ap()
    y_d = nc.dram_tensor("y", [TPC, D_OUT], F32, kind="ExternalOutput").ap()
    with tile.TileContext(nc) as tc:
        _emit(tc, x_d, w_d, ws_d, y_d)
    nc.compile()
    _CACHE[key] = nc
    return nc


_last_result = None  # BassKernelResults of the most recent run (for profiling)


def kernel(x: np.ndarray, weight: np.ndarray, trace: bool = False) -> np.ndarray:
    global _last_result
    nc = _build()
    xf = np.ascontiguousarray(x.reshape(TOK, D_IN), dtype=np.float32)
    wf = np.ascontiguousarray(weight, dtype=np.float32)
    osh = D_OUT // N_CORES
    in_maps = [
        {
            "x": xf[c * TPC:(c + 1) * TPC],
            "w": wf,
            "ws": wf[c * osh:(c + 1) * osh],
        }
        for c in range(N_CORES)
    ]
    res = run_bass_kernel_spmd(nc, in_maps, list(range(N_CORES)), trace=trace)
    _last_result = res
    y = np.concatenate([res.results[c]["y"] for c in range(N_CORES)], axis=0)
    return y.reshape(B, S, D_OUT)



# revision 20
# speedup vs baseline: 1.2028x; 1.2028x over previous
"""BitLinear (ternary-weight + 8-bit-activation quantized matmul) on 8 TRN2 cores.

Strategy: data-parallel over tokens. Each core gets 2048 of the 16384 tokens
plus the full weight matrix, computes the whole BitLinear forward for its
token shard on device, and the host concatenates the shards.

Math (must match the jax reference):
  w_scale = max(mean(|W|), 1e-6)                       (scalar)
  w_q     = clip(round(W / w_scale), -1, 1)            (ternary)
  a       = clip(max_i |x|, 1e-8, inf)                 (per token)
  x_q     = clip(round(x * 127 / a), -127, 127)        (8-bit ints)
  y       = (x_q @ w_q^T) * w_scale * a / 127

All rounding is done with the fp32 magic-number trick (v + 1.5*2^23 - 1.5*2^23
is round-to-nearest-even), so device results bit-match jnp.round. x_q (ints
<= 127) and w_q ({-1,0,1}) are exact in bf16 and products accumulate exactly
in fp32 PSUM (|y_int| < 2^24), so the bf16 TensorE matmul is exact.

Schedule (v2): single fused W pass -- the abs-mean scan stages the first
KST W tiles in SBUF so quantization starts the moment w_scale is known,
with no second read on the critical path. The GEMM opens with a 4x4
(token-tile x output-quarter) block so the first matmuls only need the
first quarter of quantized W; the quantize stream races the opening
block instead of the first 14us of GEMM. x-side prep never depends on
w_scale (the per-token output scale is folded in at PSUM-drain time), so
token tiles 0..3 are prepped during the quantize window. Engine split:
scalar = quantize IDENTITYs, DVE = clips/small chains/PSUM drains,
GpSimd = abs-max + x transposes, Sync = W transposes, sync ring = bulk
DMA.
"""

from contextlib import ExitStack

import numpy as np

import concourse.bass as bass
import concourse.tile as tile
from concourse import bacc, bass_isa, mybir
from concourse.bass import ds, ts
from concourse.bass_utils import run_bass_kernel_spmd

F32 = mybir.dt.float32
BF16 = mybir.dt.bfloat16
AF = mybir.ActivationFunctionType
OP = mybir.AluOpType
AX = mybir.AxisListType

B, S, D_IN, D_OUT = 4, 4096, 2048, 2048
N_CORES = 8
TOK = B * S                # 16384 tokens
TPC = TOK // N_CORES       # 2048 tokens per core
NT = TPC // 128            # 16 token tiles per core
NJ = D_OUT // 128          # 16 weight row tiles
NI = D_IN // 128           # 16 contraction (k) blocks
NO = D_OUT // 512          # 4 output column blocks
CM = 12582912.0            # 1.5 * 2^23: fp32 RNE rounding magic
QMAX = 127.0

KNOBS = {
    "kst": 8,            # W tiles staged in SBUF during the abs-mean scan
    "open": 4,           # token tiles in the opening (t, no) block
    "pref": 4,           # x tiles prepped before the main loop
    "ldx_bufs": 2,       # f32 x load pool
    "xqt_bufs": 4,       # transposed x tiles in flight
    "wq_bufs": 4,        # quantized W staging (pre-transpose)
    "xq_bufs": 2,
    "ys_bufs": 3,
    "wtp_eng": "y",      # W transpose engine: s=scalar(Act ring), y=sync
    "xtp_eng": "y",      # x transpose engine
    "xmax_eng": "v",     # x abs-max engine: v or g
    "xq_eng0": "vvvv",   # x quantize engine per prefetched tile
    "xq_engN": "s",      # x quantize engine steady state
    "xsm_eng0": "v",     # small-chain engine for prefetched tiles
    "xadd_eng0": "vvvv",  # -CM add engine per prefetched tile
    "p1_stage_eng": "v",  # staged-tile abs reduce engine(s)
    # full load-ring order: pN = pass-1 W tile N, xN = x tile N,
    # rN = re-read W tile KST+N. All pN tokens must precede the rws chain.
    "ring_order": (
        [f"p{j}" for j in range(8, 16)]       # destructive (re-read later)
        + ["p0", "p1", "p2", "p3", "x0", "p4", "x1", "p5", "x2", "p6",
           "x3", "p7"]                        # staged + x prefetch
        + [f"r{i}" for i in range(8)]         # re-reads of p8..15
    ),
    # startup issue program (see _emit): wN = IDENT+clip, uN = W transpose
    # trigger, XN = x chain compute, TN = x transpose trigger,
    # rN = re-read load of W tile KST+N
    "startup_prog": ["w0", "X0", "w1", "u0", "w2", "u1", "X1", "w3", "u2",
                     "T0", "w4", "u3", "X2", "T1",
                     "w5", "u4", "X3", "w6", "u5", "T2", "w7", "u6",
                     "T3", "w8", "u7", "w9", "u8",
                     "w10", "u9", "w11", "u10", "w12", "u11",
                     "w13", "u12", "w14", "u13", "w15", "u14", "u15"],
    "slot_ms": 0.003,    # designed time per load slot in pass 1
    "rws_ms": 0.050,     # designed time of the w_scale reduce chain
    # designed times for startup_prog tokens (ms); tokens absent = no pin
    "prog_ms": (
        {f"w{j}": 0.052 + 0.002 * j for j in range(16)}
        | {f"u{j}": 0.0565 + 0.002 * j for j in range(16)}
        | {f"X{t}": 0.030 + 0.006 * t for t in range(4)}
        | {f"T{t}": 0.054 + 0.004 * t for t in range(4)}
        | {f"r{i}": 0.058 + 0.003 * i for i in range(8)}
    ),
}

_CACHE = {}


def _emit(tc: tile.TileContext, x_d: bass.AP, w_d: bass.AP, y_d: bass.AP):
    nc = tc.nc
    KST = KNOBS["kst"]
    OPEN = KNOBS["open"]
    PREF = KNOBS["pref"]

    def eng(c):
        return {"s": nc.scalar, "v": nc.vector, "g": nc.gpsimd, "y": nc.sync}[c]

    with ExitStack() as ctx:
        ld = ctx.enter_context(tc.tile_pool(name="ld", bufs=2))
        ldx = ctx.enter_context(tc.tile_pool(name="ldx", bufs=KNOBS["ldx_bufs"]))
        wstp = ctx.enter_context(tc.tile_pool(name="wstp", bufs=1))
        wqp = ctx.enter_context(tc.tile_pool(name="wqp", bufs=KNOBS["wq_bufs"]))
        xqp = ctx.enter_context(tc.tile_pool(name="xqp", bufs=KNOBS["xq_bufs"]))
        xqtp = ctx.enter_context(tc.tile_pool(name="xqtp", bufs=KNOBS["xqt_bufs"]))
        wqtp = ctx.enter_context(tc.tile_pool(name="wqtp", bufs=1))
        ysp = ctx.enter_context(tc.tile_pool(name="ysp", bufs=KNOBS["ys_bufs"]))
        stats = ctx.enter_context(tc.tile_pool(name="stats", bufs=6))
        consts = ctx.enter_context(tc.tile_pool(name="consts", bufs=1))
        psum = ctx.enter_context(
            tc.tile_pool(name="psum", bufs=8, space=bass.MemorySpace.PSUM)
        )

        cpos = consts.tile([128, 1], F32, tag="cpos")
        nc.vector.memset(cpos, CM)
        czero = consts.tile([128, 1], F32, tag="czero")
        nc.vector.memset(czero, 0.0)
        # per-token-tile stats, one column per tile: a (clipped absmax) and
        # sout (a * w_scale / 127, filled in lazily once w_scale exists)
        amat = consts.tile([128, NT], F32, tag="amat")
        soutmat = consts.tile([128, NT], F32, tag="soutmat")

        # ---- fused pass 1: abs-sum of W + stage the first KST tiles ----
        # Tiles j >= KST (the LAST output quarters, whose GEMM deadline is
        # late) go first in the DMA stream: destructive in-place Abs on the
        # scalar engine (accum_out row-sum); data is discarded and re-read
        # later. Tiles j < KST (quarters 0..) land last and stay resident, so
        # quantization starts at scalar-engine pace the moment w_scale is
        # known -- no DMA on the quarter-0 critical path.
        wsums = consts.tile([128, NJ], F32, tag="wsums")
        wst_tiles = {}

        def w_p1(jj):
            if jj < KST:
                wt = wstp.tile([128, D_IN], F32, tag=f"wst{jj}", name=f"wst{jj}")
                nc.sync.dma_start(wt, w_d[ts(jj, 128), :])
                e = KNOBS["p1_stage_eng"][jj % len(KNOBS["p1_stage_eng"])]
                eng(e).reduce_sum(
                    wsums[:, ds(jj, 1)], wt, axis=AX.X, apply_absolute_value=True
                )
                wst_tiles[jj] = wt
            else:
                wt = ld.tile([128, D_IN], F32, tag="ld", name=f"wp1_{jj}")
                nc.sync.dma_start(wt, w_d[ts(jj, 128), :])
                nc.scalar.activation(
                    wt, wt, AF.Abs, bias=czero, accum_out=wsums[:, ds(jj, 1)]
                )

        # ---- x prep: load, per-token scale, quantize, transpose ----
        # Entirely independent of w_scale (the output scale a*ws/127 is
        # applied at PSUM drain), so prefetched tiles run under pass 1.
        def x_load(t):
            xt = ldx.tile([128, D_IN], F32, tag="ldx", name=f"xt{t}")
            nc.sync.dma_start(xt, x_d[ts(t, 128), :])
            return xt

        def x_chain(t, xt, qe, sme):
            eng(KNOBS["xmax_eng"]).reduce_max(
                amat[:, ds(t, 1)], xt, axis=AX.X, apply_absolute_value=True
            )
            a = amat[:, ds(t, 1)]
            sm = eng(sme)
            sm.tensor_scalar(a, a, 1e-8, None, OP.max)
            r0 = stats.tile([128, 1], F32, tag="xr0", name=f"xr0{t}")
            sm.reciprocal(r0, a)
            ntt = stats.tile([128, 1], F32, tag="xntt", name=f"xntt{t}")
            sm.tensor_mul(ntt, a, r0)
            sm.tensor_scalar(ntt, ntt, -1.0, 2.0, OP.mult, OP.add)
            s = stats.tile([128, 1], F32, tag="xs", name=f"xs{t}")
            sm.tensor_mul(s, r0, ntt)
            sm.tensor_scalar(s, s, QMAX, None, OP.mult)  # 127/a

            # t1 = x*s + CM in place (fp32 add at ulp=1 == RNE round)
            if qe == "s":
                nc.scalar.activation(xt, xt, AF.Identity, bias=cpos, scale=s)
            else:
                eng(qe).tensor_scalar(xt, xt, s, CM, OP.mult, OP.add)
            xq = xqp.tile([128, D_IN], BF16, tag="xq", name=f"xq{t}")
            ae = KNOBS["xadd_eng0"][t] if t < PREF else "v"
            eng(ae).tensor_scalar(xq, xt, -CM, None, OP.add)
            return xq

        def x_tpose_x(t, xq):
            xqT = xqtp.tile([128, NI, 128], BF16, tag="xqT", name=f"xqT{t}")
            eng(KNOBS["xtp_eng"]).dma_start(xqT, xq, transpose=True)
            return xqT

        def x_prep(t, qe, sme="v"):
            return x_tpose_x(t, x_chain(t, x_load(t), qe, sme))

        wre_tiles = {}
        xts = {}

        def w_reload(jj):
            # recycle the staged-tile buffer freed by quantize of jj-KST
            wt = wstp.tile([128, D_IN], F32, tag=f"wst{jj % KST}", name=f"wre{jj}")
            nc.sync.dma_start(wt, w_d[ts(jj, 128), :])
            wre_tiles[jj] = wt

        ring = list(KNOBS["ring_order"])
        # pass-1 tokens (pN) must all precede the rws chain; emit them (plus
        # any interleaved x loads) now, remember the rest for after.
        # Each load slot gets a designed schedule time (tile_wait_until) so
        # the static scheduler reproduces the intended pipeline.
        SLOT = KNOBS["slot_ms"]
        last_p = max(i for i, tok in enumerate(ring) if tok[0] == "p")
        for i, tok in enumerate(ring[: last_p + 1]):
            kind, idx = tok[0], int(tok[1:])
            with tc.tile_wait_until(i * SLOT):
                if kind == "p":
                    w_p1(idx)
                elif kind == "x":
                    xts[idx] = x_load(idx)
                else:
                    w_reload(KST + idx)
        ring_rest = ring[last_p + 1:]

        wsum_p = stats.tile([128, 1], F32, tag="wsp")
        ctx.enter_context(tc.tile_wait_until(KNOBS["rws_ms"]))
        nc.vector.reduce_sum(wsum_p, wsums, axis=AX.X)
        wsum_all = stats.tile([128, 1], F32, tag="wsa")
        nc.gpsimd.partition_all_reduce(wsum_all, wsum_p, 128, bass_isa.ReduceOp.add)
        # w_scale = max(sum / (O*I), 1e-6)
        wscale = consts.tile([128, 1], F32, tag="wscale")
        nc.vector.tensor_scalar(
            wscale, wsum_all, 1.0 / (D_OUT * D_IN), 1e-6, OP.mult, OP.max
        )
        # rws ~= 1/w_scale with one Newton refinement
        r0 = stats.tile([128, 1], F32, tag="wr0")
        nc.vector.reciprocal(r0, wscale)
        ntt = stats.tile([128, 1], F32, tag="wntt")
        nc.vector.tensor_mul(ntt, wscale, r0)
        nc.vector.tensor_scalar(ntt, ntt, -1.0, 2.0, OP.mult, OP.add)
        rws = consts.tile([128, 1], F32, tag="rws")
        nc.vector.tensor_mul(rws, r0, ntt)
        ws127 = consts.tile([128, 1], F32, tag="ws127")
        nc.vector.tensor_scalar(ws127, wscale, 1.0 / QMAX, None, OP.mult)


        # ---- W quantize: scalar IDENTITY + DVE clip (bf16 domain) ----
        # wqT[no][i_in, jq, i_blk, o_in] = w_q[(no*4+jq)*128 + o_in, i_blk*128 + i_in]
        wqT = [
            wqtp.tile([128, NJ // NO, NI, 128], BF16, tag=f"wqT{no}", name=f"wqT{no}")
            for no in range(NO)
        ]

        wqs = {}

        def w_quant(j):
            src = wst_tiles[j] if j < KST else wre_tiles[j]
            # t1 = W*rws + CM in place
            nc.scalar.activation(src, src, AF.Identity, bias=cpos, scale=rws)
            wq = wqp.tile(
                [128, D_IN], BF16, tag=f"wqn{j % KNOBS['wq_bufs']}",
                name=f"wq{j}", bufs=1,
            )
            # (t1 - CM) min 1 -> bf16 (exact small ints), then max -1 in bf16
            nc.vector.tensor_scalar(wq, src, -CM, 1.0, OP.add, OP.min)
            nc.vector.tensor_scalar(wq, wq, -1.0, None, OP.max)
            wqs[j] = wq

        def w_tpose(j):
            eng(KNOBS["wtp_eng"]).dma_start(
                wqT[j // 4][:, j % 4, :, :], wqs[j], transpose=True
            )

        for tok in ring_rest:
            kind, idx = tok[0], int(tok[1:])
            if kind == "x":
                xts[idx] = x_load(idx)
            else:
                w_reload(KST + idx)

        # startup issue program: wN = quantize W tile N, XN = x-chain compute
        # for prefetched tile N, TN = its transpose issue. Token order tunes
        # the in-order per-engine queues (scalar IDENTs, DVE clips, ACT-ring
        # transpose issues) against the quarter deadlines of the opening
        # GEMM block.
        xqTs = {}
        xqs = {}
        for tok in KNOBS["startup_prog"]:
            kind, idx = tok[0], int(tok[1:])
            ms = KNOBS["prog_ms"].get(tok, None)
            with tc.tile_wait_until(ms if ms is not None else 0,
                                    enable=ms is not None):
                if kind == "w":
                    w_quant(idx)
                elif kind == "u":
                    w_tpose(idx)
                elif kind == "r":
                    w_reload(KST + idx)
                elif kind == "X":
                    xqs[idx] = x_chain(
                        idx, xts[idx], KNOBS["xq_eng0"][idx], KNOBS["xsm_eng0"]
                    )
                else:
                    xqTs[idx] = x_tpose_x(idx, xqs[idx])

        # ---- GEMM ----
        # opening block: (t, no) pairs column-major over t=0..OPEN-1 so the
        # first matmuls need only wqT quarter 0; remaining quarters stream in
        # behind. Then plain token-major for the rest.
        pairs = [(t, no) for no in range(NO) for t in range(OPEN)]
        pairs += [(t, no) for t in range(OPEN, NT) for no in range(NO)]

        prep_next = PREF
        sout_done = set()
        for i, (t, no) in enumerate(pairs):
            # software-pipelined x prep: one tile every 4 pairs
            if i % NO == 0 and prep_next < NT:
                xqTs[prep_next] = x_prep(prep_next, KNOBS["xq_engN"])
                prep_next += 1

            if t not in sout_done:
                nc.vector.tensor_mul(
                    soutmat[:, ds(t, 1)], amat[:, ds(t, 1)], ws127
                )
                sout_done.add(t)

            xqT = xqTs[t]
            ps = psum.tile([128, 512], F32, tag="ps")
            for b in range(NI):
                nc.tensor.matmul(
                    ps,
                    xqT[:, b, :],
                    wqT[no][:, :, b, :],
                    start=(b == 0),
                    stop=(b == NI - 1),
                )
            yt = ysp.tile([128, 512], F32, tag="ys")
            nc.vector.tensor_scalar(yt, ps, soutmat[:, ds(t, 1)], None, OP.mult)
            nc.sync.dma_start(y_d[ts(t, 128), ts(no, 512)], yt)


def _build():
    key = tuple(sorted((k, str(v)) for k, v in KNOBS.items()))
    if key in _CACHE:
        return _CACHE[key]
    nc = bacc.Bacc(
        "TRN2", target_bir_lowering=False, debug=False, num_devices=N_CORES
    )
    x_d = nc.dram_tensor("x", [TPC, D_IN], F32, kind="ExternalInput").ap()
    w_d = nc.dram_tensor("w", [D_OUT, D_IN], F32, kind="ExternalInput").ap()
    y_d = nc.dram_tensor("y", [TPC, D_OUT], F32, kind="ExternalOutput").ap()
    with tile.TileContext(nc) as tc:
        _emit(tc, x_d, w_d, y_d)
    nc.compile()
    _CACHE[key] = nc
    return nc


_last_result = None  # BassKernelResults of the most recent run (for profiling)


def kernel(x: np.ndarray, weight: np.ndarray, trace: bool = False) -> np.ndarray:
    global _last_result
    nc = _build()
    xf = np.ascontiguousarray(x.reshape(TOK, D_IN), dtype=np.float32)
    wf = np.ascontiguousarray(weight, dtype=np.float32)
    in_maps = [
        {"x": xf[c * TPC:(c + 1) * TPC], "w": wf}
        for c in range(N_CORES)
    ]
    res = run_bass_kernel_spmd(nc, in_maps, list(range(N_CORES)), trace=trace)
    _last_result = res
    y = np.concatenate([res.results[c]["y"] for c in range(N_CORES)], axis=0)
    return y.reshape(B, S, D_OUT)
